# revision 1
# baseline (speedup 1.0000x reference)
"""Trainium2 Bass kernel for nn_DemandMap (histogram_binning).

Math: the scatter-add histogram is a dense separable 8x8 block reduction.
Site (i,j) of type t contributes ox(t, i%8)*oy(t, j%8) area terms to bins
(i//8 (+1), j//8 (+1)), so per type:  hist_t = WX_t^T @ mask_t @ WY_t
with banded weight matrices that depend only on (residue, type).

Device dataflow per core (j on partitions, i on free axis):
  smapT[jp, r] int16 (j zero-padded by 8 at top) --DMA--> [128, 512] tiles;
  35 j-tiles at stride 120 (8-row y-halo) so each tile fully owns 15 y-bins.
  masks: m_t = (st == t+1) in bf16          (DVE tensor_scalar + gpsimd)
  stage A (y-reduce): PE matmul psA[32t + (0..29), i] = WY_t^T @ m_t,
       WY_t = [128, 32] bf16 = 15 hi + 15 lo + 2 zero cols (hi/lo split of
       the f32 weights keeps fp32-level accuracy at bf16 matmul speed)
  drain psA [96, 512] f32 -> AD[g] [96, 1536] bf16 (ScalarE copy, cast)
  one xbar transpose per 3-tile group: [96, 1536] -> [128, 12, 96] into a
       per-4-group ATC tensor (so stage B sees a uniform 384-stride layout)
  stage B (x-reduce): PE matmul psB[65, 30*JL+q] += WXhi^T @ ATC
       (65th row = spill into the next core's first bin; replaces an x-halo)
       + WXlo^T @ hi-cols onto lo-cols when weights are not bf16-exact
  merge hi+lo (DVE), assemble 64-hist via ScalarE Copy(scale=-1, bias=64),
  DMA out [4, 65, 512].

Sharding: core c owns bins bx in [64c, 64c+64) and reads site rows
[512c, 512c+512) only.  The kx=1 spill that crosses the core boundary is
returned as output row 64 and added into the next core's first bin column
on the host ((64-h0) + (64-h1) - 64 = 64 - h0 - h1).
"""

import json
import os

import numpy as np
import ml_dtypes

BF16 = ml_dtypes.bfloat16

NCORES = 8
RPAD = 512  # site rows per core (i axis) — exactly 4 transpose chunks
IPAD = 512
JPAD = 4224  # 8 top zero pad + 4096 + tail pad, j axis (partition source)
NT = 35  # j-tiles, stride 120, each owns 15 y-bins
GROUPS = [list(range(6 * g, 6 * g + 6)) for g in range(4)] + [
    [24, 25, 26, 27, 28],
    [29, 30, 31],
    [32, 33, 34],
]
GCHUNKS = [[0, 1], [2, 3], [4, 5, 6]]
# JL offset of each group inside its chunk's ATC tensor
GOFF = {0: 0, 1: 6, 2: 0, 3: 6, 4: 0, 5: 5, 6: 8}
NAD = int(os.environ.get("KERNEL_NAD", "4"))  # rotating A-drain buffers
NIN_DMA = int(os.environ.get("KERNEL_NIN", "6"))  # input loaded in this many big DMAs


def _nbins(T):
    return 15 if T < 34 else 2


_PROG_CACHE = {}
_WSPLIT_DONE = [False]


def _install_wait_split():
    """This walrus build accepts only ONE sync wait per instruction; Tile
    attaches N.  Rewrite the BIR JSON: hoist all-but-one wait onto fresh
    same-engine EventSemaphore waits inserted before the offender."""
    if _WSPLIT_DONE[0]:
        return
    import concourse.bass as bass

    orig = bass.Bass.to_json_bytes

    def split(self, *a, **k):
        m = json.loads(orig(self, *a, **k))
        n = [0]
        for fn in m["functions"]:
            for blk in fn["blocks"]:
                out = []
                for ins in blk["instructions"]:
                    si = ins.get("sync_info")
                    waits = si.get("on_wait") if si else None
                    if waits and len(waits) > 1:
                        for w in waits[:-1]:
                            n[0] += 1
                            nop = {
                                "engine": ins["engine"],
                                "ins": [],
                                "outs": [],
                                "name": f"WSPLIT-{n[0]}",
                                "opcode": "EventSemaphore",
                                "sync_info": {"on_update": [], "on_wait": [w]},
                            }
                            if "debug" in ins:
                                nop["debug"] = ins["debug"]
                            out.append(nop)
                        si["on_wait"] = [waits[-1]]
                    out.append(ins)
                blk["instructions"] = out
        return json.dumps(m).encode()

    bass.Bass.to_json_bytes = split
    _WSPLIT_DONE[0] = True


def _oxy_weights(size_f32):
    """Per-residue overlap weights, matching the reference f32 formulas."""
    r = np.arange(8, dtype=np.float32)
    o0 = np.maximum(np.minimum(np.float32(8.0) - r, size_f32), np.float32(0.0))
    o1 = np.maximum(
        np.minimum(np.float32(16.0) - r, size_f32) - (np.float32(8.0) - r),
        np.float32(0.0),
    )
    return o0.astype(np.float32), o1.astype(np.float32)


def _build_wy(sy):
    """f32 [128, 3, 15]: y-stage stationary (y-halo tile form), types 1..3.
    Tile partition p holds padded j = 120T + p (jp = j + 8), so ry = p % 8;
    bin q gets ky=0 rows p//8 == q+1 and ky=1 rows p//8 == q."""
    W = np.zeros((128, 3, 15), np.float32)
    for tp in range(3):
        o0, o1 = _oxy_weights(np.float32(sy[tp + 1]))
        for p in range(128):
            if 0 <= p // 8 - 1 < 15:
                W[p, tp, p // 8 - 1] += o0[p % 8]
            if p // 8 < 15:
                W[p, tp, p // 8] += o1[p % 8]
    return W


def _build_wx(sx):
    """f32 [IPAD, 3, 65]: x-stage stationary.  Local row r = i - 512c; col 64
    is the spill bin (kx=1 of the last 8 rows -> next core's first bin)."""
    W = np.zeros((IPAD, 3, 65), np.float32)
    for tp in range(3):
        o0, o1 = _oxy_weights(np.float32(sx[tp + 1]))
        for r in range(512):
            W[r, tp, r // 8] += o0[r % 8]
            W[r, tp, r // 8 + 1] += o1[r % 8]
    return W


def _hi_lo(w):
    hi = w.astype(BF16)
    lo = (w - hi.astype(np.float32)).astype(BF16)
    return hi, lo


def _build_program(use_xlo, use_ylo):
    _install_wait_split()
    import os as _os
    import concourse.bass as bass
    import concourse.mybir as mybir
    from concourse.tile import TileContext
    from contextlib import ExitStack

    bufs_st = int(_os.environ.get("KERNEL_BUFS_ST", "6"))
    bufs_m = int(_os.environ.get("KERNEL_BUFS_M", "4"))
    bufs_pa = int(_os.environ.get("KERNEL_BUFS_PA", "3"))
    nmask_dve = int(_os.environ.get("KERNEL_MASK_DVE", "2"))
    ndrain_dve = int(_os.environ.get("KERNEL_DRAIN_DVE", "0"))

    dt = mybir.dt
    nc = bass.Bass()
    # host-packed tiles: smapT[p, 512*T + r] = site_type[120*T + p - 8, r]
    smapT = nc.declare_dram_parameter(
        "smapT", [128, NT * RPAD], dt.int16, isOutput=False
    )
    # all weights in one tensor/DMA: wy cols 0:96, wxh[t][k] at 96+65*(4t+k),
    # wxl (if used) at 876+65*(4t+k)
    WTOT = 96 + 780 + (780 if use_xlo else 0)
    WPACK = nc.declare_dram_parameter(
        "wpack", [128, WTOT], dt.bfloat16, isOutput=False
    )
    OUT = nc.declare_dram_parameter("outbuf", [3, 65, 512], dt.float32, isOutput=True)

    merge_lo = use_xlo or use_ylo
    RW = 30 if use_ylo else 15  # stage-B rhs width: lo cols are zero w/o ylo
    NCH = [sum(len(GROUPS[g]) for g in gc) for gc in GCHUNKS]  # JL per chunk

    with ExitStack() as ctx:
        tc = ctx.enter_context(TileContext(nc))
        # ---- pools ------------------------------------------------------
        pp = ctx.enter_context(tc.tile_pool(name="persist", bufs=1))
        mp = ctx.enter_context(tc.tile_pool(name="masks", bufs=bufs_m))
        if use_xlo:
            bufs_pa = min(bufs_pa, 2)  # psA is 2 banks now; keep total <= 8
        pA = ctx.enter_context(tc.tile_pool(name="psA", bufs=bufs_pa, space="PSUM"))
        pB = ctx.enter_context(tc.tile_pool(name="psB", bufs=2, space="PSUM"))
        pB2 = (
            ctx.enter_context(tc.tile_pool(name="psB2", bufs=2, space="PSUM"))
            if use_xlo
            else None
        )

        # ---- persistent SBUF tensors ------------------------------------
        wpack = pp.tile([128, WTOT], dt.bfloat16, name="wpack", tag="wpack")
        wy = wpack[:, 0:96]
        wxh = [
            [wpack[:, 96 + 65 * (4 * t + k) : 96 + 65 * (4 * t + k) + 65] for k in range(4)]
            for t in range(3)
        ]
        wxl = (
            [
                [
                    wpack[:, 876 + 65 * (4 * t + k) : 876 + 65 * (4 * t + k) + 65]
                    for k in range(4)
                ]
                for t in range(3)
            ]
            if use_xlo
            else None
        )
        AD = [
            pp.tile([96, 6 * IPAD], dt.bfloat16, name=f"ad_{i}", tag=f"ad_{i}")
            for i in range(NAD)
        ]
        # per-chunk transposed A: col(JL, k, q) = 384*JL + 96*k + q
        ATC = [
            pp.tile([128, 384 * 12], dt.bfloat16, name=f"atc_{cc}", tag=f"atc_{cc}")
            for cc in range(len(GCHUNKS))
        ]
        stbig = pp.tile([128, NT * RPAD], dt.int16, name="stbig", tag="stbig")
        outp = [
            pp.tile([65, 512], dt.float32, name=f"outp_{t}", tag=f"outp_{t}")
            for t in range(3)
        ]

        # ---- preload weights: one DMA -----------------------------------
        nc.sync.dma_start(out=wpack[:, :], in_=WPACK[:, :])

        # ---- phase 1: graded input DMAs (small first so masks start early)
        sizes = [int(x) for x in _os.environ.get("KERNEL_GRADE", "2,4,6").split(",")]
        while sum(sizes) < NT:
            sizes.append(min(8, NT - sum(sizes)))
        pos = 0
        for sz in sizes:
            c0 = pos * RPAD
            c1 = (pos + sz) * RPAD
            nc.sync.dma_start(out=stbig[:, c0:c1], in_=smapT[:, c0:c1])
            pos += sz
        drain_i = 0
        for g, Ts in enumerate(GROUPS):
            ad = AD[g % NAD]
            cc = next(i for i, gc in enumerate(GCHUNKS) if g in gc)
            gl = GCHUNKS[cc].index(g)
            for pr in range(0, len(Ts), 2):
                subs = [s for s in (0, 1) if pr + s < len(Ts)]
                psA = pA.tile([96, 1024], dt.float32)  # two tiles, bank-aligned
                for s in subs:
                    jl = pr + s
                    T = Ts[jl]
                    st16 = stbig[:, RPAD * T : RPAD * T + RPAD]
                    masks = []
                    for t in range(3):
                        m = mp.tile([128, RPAD], dt.bfloat16, tag=f"m{t}")
                        use_dve = t < nmask_dve or (t == nmask_dve and (T % 2) == 0)
                        eng = nc.vector if use_dve else nc.gpsimd
                        eng.tensor_scalar(
                            m[:, :],
                            st16,
                            float(t + 1),
                            None,
                            mybir.AluOpType.is_equal,
                        )
                        masks.append(m)
                    for t in range(3):
                        nc.tensor.matmul(
                            psA[32 * t : 32 * t + 32, 512 * s : 512 * s + 512],
                            lhsT=wy[:, 32 * t : 32 * t + 32],
                            rhs=masks[t][:, :],
                            start=True,
                            stop=True,
                        )
                w = 512 * len(subs)
                if (drain_i % 3) < ndrain_dve:
                    nc.vector.tensor_copy(
                        out=ad[0:96, IPAD * pr : IPAD * pr + w], in_=psA[:, 0:w]
                    )
                else:
                    nc.scalar.copy(ad[0:96, IPAD * pr : IPAD * pr + w], psA[:, 0:w])
                drain_i += 1
            # one xbar transpose per group (variable tile count)
            nT = len(Ts)
            jlo = GOFF[g]
            nc.sync.dma_start_transpose(
                ATC[cc][:, 384 * jlo : 384 * jlo + 384 * nT].rearrange(
                    "p (c q) -> p c q", q=96
                ),
                ad[0:96, 0 : nT * IPAD],
            )

        # ---- phase 2: x-reduce + assembly (chunk-major: the last chunk
        # depends on the last transpose, so it must come last on the PE) ----
        for cc, gc in enumerate(GCHUNKS):
            for t in range(3):
                njl = NCH[cc]
                psB = pB.tile([65, 512], dt.float32)
                for k in range(4):
                    rhs = ATC[cc][:, 0 : 384 * njl].rearrange(
                        "p (jl four q) -> p jl four q", four=4, q=96
                    )[:, :, k, 32 * t : 32 * t + RW]
                    nc.tensor.matmul(
                        psB[:, 0 : RW * njl],
                        lhsT=wxh[t][k],
                        rhs=rhs,
                        start=(k == 0),
                        stop=(k == 3),
                    )
                psB2 = None
                if use_xlo:
                    psB2 = pB2.tile([65, 512], dt.float32)
                    for k in range(4):
                        rhs = ATC[cc][:, 0 : 384 * njl].rearrange(
                            "p (jl four q) -> p jl four q", four=4, q=96
                        )[:, :, k, 32 * t : 32 * t + 15]
                        nc.tensor.matmul(
                            psB2[:, 0 : 15 * njl],
                            lhsT=wxl[t][k],
                            rhs=rhs,
                            start=(k == 0),
                            stop=(k == 3),
                        )
                reg = psB[:, 0 : RW * njl].rearrange("p (jl c) -> p jl c", c=RW)
                if use_ylo:
                    nc.vector.scalar_tensor_tensor(
                        out=reg[:, :, 0:15],
                        in0=reg[:, :, 0:15],
                        scalar=1.0,
                        in1=reg[:, :, 15:30],
                        op0=mybir.AluOpType.mult,
                        op1=mybir.AluOpType.add,
                    )
                if use_xlo:
                    nc.vector.scalar_tensor_tensor(
                        out=reg[:, :, 0:15],
                        in0=reg[:, :, 0:15],
                        scalar=1.0,
                        in1=psB2[:, 0 : 15 * njl].rearrange(
                            "p (jl c) -> p jl c", c=15
                        ),
                        op0=mybir.AluOpType.mult,
                        op1=mybir.AluOpType.add,
                    )
                # out[:, 15*T + q] = 64 - psB[JL, q<15]
                T0 = GROUPS[gc[0]][0]
                nfull = njl if cc < 2 else njl - 1
                nc.scalar.activation(
                    outp[t][:, 15 * T0 : 15 * T0 + 15 * nfull],
                    reg[:, 0:nfull, 0:15],
                    mybir.ActivationFunctionType.Copy,
                    bias=64.0,
                    scale=-1.0,
                )
                if cc == 2:  # T=34 tail: only bins 510, 511
                    nc.scalar.activation(
                        outp[t][:, 510:512],
                        reg[:, njl - 1, 0:2],
                        mybir.ActivationFunctionType.Copy,
                        bias=64.0,
                        scale=-1.0,
                    )
                    nc.sync.dma_start(out=OUT[t, :, :], in_=outp[t][:, :])
    return nc


def _get_program(use_xlo, use_ylo):
    key = (use_xlo, use_ylo)
    if key not in _PROG_CACHE:
        _PROG_CACHE[key] = _build_program(use_xlo, use_ylo)
    return _PROG_CACHE[key]


def kernel(site_type_map, site_size_x, site_size_y):
    from concourse.bass_utils import run_bass_kernel_spmd

    smap = np.asarray(site_type_map, dtype=np.int32)
    sx = np.asarray(site_size_x, dtype=np.float32)
    sy = np.asarray(site_size_y, dtype=np.float32)

    WYf = _build_wy(sy)  # [128, 3, 15]
    WXf = _build_wx(sx)  # [IPAD, 3, 65]
    wy_hi, wy_lo = _hi_lo(WYf)
    wx_hi, wx_lo = _hi_lo(WXf)
    use_ylo = bool(np.any(wy_lo.astype(np.float32) != 0))
    use_xlo = bool(np.any(wx_lo.astype(np.float32) != 0))

    # WY device layout [128, 96]: per type 15 hi, 15 lo, 2 zero pad cols
    WTOT = 96 + 780 + (780 if use_xlo else 0)
    wpk = np.zeros((128, WTOT), BF16)
    for t in range(3):
        wpk[:, 32 * t : 32 * t + 15] = wy_hi[:, t, :]
        wpk[:, 32 * t + 15 : 32 * t + 30] = wy_lo[:, t, :]
        for k in range(4):
            o = 96 + 65 * (4 * t + k)
            wpk[:, o : o + 65] = wx_hi[128 * k : 128 * k + 128, t, :]
            if use_xlo:
                o2 = 876 + 65 * (4 * t + k)
                wpk[:, o2 : o2 + 65] = wx_lo[128 * k : 128 * k + 128, t, :]

    nc = _get_program(use_xlo, use_ylo)

    in_maps = []
    for c in range(NCORES):
        sjp = np.zeros((JPAD, RPAD), np.int16)
        sjp[8 : 8 + 4096, :] = smap[512 * c : 512 * c + 512].T
        big = np.empty((128, NT * RPAD), np.int16)
        for T in range(NT):
            big[:, RPAD * T : RPAD * T + RPAD] = sjp[120 * T : 120 * T + 128, :]
        m = {"smapT": big, "wpack": wpk}
        in_maps.append(m)

    res = run_bass_kernel_spmd(
        nc,
        in_maps,
        core_ids=list(range(NCORES)),
        trace=bool(int(os.environ.get("KERNEL_TRACE", "0"))),
    )
    kernel._last_results = res

    # device returns 3 type planes; comp2site=(1,1,2,3) duplicates plane 0
    full = np.empty((4, 512, 512), np.float32)
    for c in range(NCORES):
        ob = res.results[c]["outbuf"]
        full[1:4, 64 * c : 64 * c + 64, :] = ob[:, 0:64, :]
    for c in range(NCORES - 1):
        # spill row: (64-h0) + (64-h1) - 64 = 64 - h0 - h1
        full[1:4, 64 * (c + 1), :] += res.results[c]["outbuf"][:, 64, :] - np.float32(
            64.0
        )
    full[0] = full[1]
    return full



# revision 5
# speedup vs baseline: 1.1535x; 1.1535x over previous
"""Trainium2 Bass kernel for nn_DemandMap (histogram_binning).

Math: the scatter-add histogram is a dense separable 8x8 block reduction:
hist_t = WX_t^T @ mask_t @ WY_t with banded weights by (residue, type).

This version computes stage A *transposed* on the PE so no DMA transpose or
wide PSUM drains are needed:

  input: 3 mask bits per site packed as nibbles, 4 sites per int16 word
         (0.5 B/site).  word n of j-tile T holds bit (4s + t-1) =
         [site_type(x=128s+n, j) == t] for slot s (= x-chunk), type t.
  masks: DVE 2-op bitvec chains (shift to bit 10, AND 0x0400) produce int16
         tiles whose fp16 bitcast is exactly 2^-14 * mask (4x DVE mode since
         all operands are 2-byte).  wy weights are pre-scaled by 2^14 (exact).
         The last POOL_TILES j-tiles instead get is_equal masks (value 1.0)
         on the gpsimd engine from a small raw site-type side plane, with
         unscaled wy.
  stage A-T: psAT[i, 15t+q] = mask_t[:, chunk].T @ wy_t  -- the mask is the
         stationary operand (weight load) and the moving operand is only 15
         columns, so the PE cost is tiny; output is already [x, (t,q)].
  drain: ACT copies psAT f32 -> fp16 AT (exact: values are 0.5-step <= 40).
  stage B: psB[65, (T,t,q)] += wx_c^T @ AT_c accumulated over the 4 x-chunks
         (contraction over x within the chunk happens inside the matmul).
  assembly: ACT writes 64 - psB into the output planes; row 64 is the kx=1
         spill into the next core's first x-bin, merged on the host.

Sharding: core c owns x rows [512c, 512c+512) and bins bx in [64c, 64c+64).
"""

import json
import os

import numpy as np

F16 = np.float16

NCORES = 8
RPAD = 512   # site x cols per core
JPAD = 4224  # 8 top zero pad + 4096 + tail pad (y axis, partition source)
NT = 35      # j-tiles, stride 120, each owns 15 y-bins
NTYPES = 3
MBIT = 1 << 10       # surviving mask bit -> fp16 2^-14
MVAL = 2.0 ** -14
WYSCALE = 2.0 ** 14

POOL_TILES = int(os.environ.get("KERNEL_POOL_TILES", "5"))
ROUND_LEN = int(os.environ.get("KERNEL_ROUND_LEN", "9"))
NIN_DMA = int(os.environ.get("KERNEL_NIN", "3"))

_PROG_CACHE = {}
_WSPLIT_DONE = [False]


def _install_wait_split():
    """This walrus build accepts only ONE sync wait per instruction; Tile
    attaches N.  Rewrite the BIR JSON: hoist all-but-one wait onto fresh
    same-engine EventSemaphore waits inserted before the offender."""
    if _WSPLIT_DONE[0]:
        return
    import concourse.bass as bass

    orig = bass.Bass.to_json_bytes

    def split(self, *a, **k):
        m = json.loads(orig(self, *a, **k))
        n = [0]
        for fn in m["functions"]:
            for blk in fn["blocks"]:
                out = []
                for ins in blk["instructions"]:
                    si = ins.get("sync_info")
                    waits = si.get("on_wait") if si else None
                    if waits and len(waits) > 1:
                        for w in waits[:-1]:
                            n[0] += 1
                            nop = {
                                "engine": ins["engine"],
                                "ins": [],
                                "outs": [],
                                "name": f"WSPLIT-{n[0]}",
                                "opcode": "EventSemaphore",
                                "sync_info": {"on_update": [], "on_wait": [w]},
                            }
                            if "debug" in ins:
                                nop["debug"] = ins["debug"]
                            out.append(nop)
                        si["on_wait"] = [waits[-1]]
                    out.append(ins)
                blk["instructions"] = out
        return json.dumps(m).encode()

    bass.Bass.to_json_bytes = split
    _WSPLIT_DONE[0] = True


def _oxy_weights(size_f32):
    """Per-residue overlap weights, matching the reference f32 formulas."""
    r = np.arange(8, dtype=np.float32)
    o0 = np.maximum(np.minimum(np.float32(8.0) - r, size_f32), np.float32(0.0))
    o1 = np.maximum(
        np.minimum(np.float32(16.0) - r, size_f32) - (np.float32(8.0) - r),
        np.float32(0.0),
    )
    return o0.astype(np.float32), o1.astype(np.float32)


def _build_wy(sy):
    """f32 [128, 3, 15]: y-stage stationary (y-halo tile form), types 1..3.
    Tile partition p holds padded j = 120T + p (jp = j + 8), so ry = p % 8;
    bin q gets ky=0 rows p//8 == q+1 and ky=1 rows p//8 == q."""
    W = np.zeros((128, 3, 15), np.float32)
    for tp in range(3):
        o0, o1 = _oxy_weights(np.float32(sy[tp + 1]))
        for p in range(128):
            if 0 <= p // 8 - 1 < 15:
                W[p, tp, p // 8 - 1] += o0[p % 8]
            if p // 8 < 15:
                W[p, tp, p // 8] += o1[p % 8]
    return W


def _build_wx(sx):
    """f32 [512, 3, 65]: x-stage stationary.  Local row r = x - 512c; col 64
    is the spill bin (kx=1 of the last 8 rows -> next core's first bin)."""
    W = np.zeros((RPAD, 3, 65), np.float32)
    for tp in range(3):
        o0, o1 = _oxy_weights(np.float32(sx[tp + 1]))
        for r in range(512):
            W[r, tp, r // 8] += o0[r % 8]
            W[r, tp, r // 8 + 1] += o1[r % 8]
    return W


def _hi_lo16(w):
    hi = w.astype(F16)
    lo = (w - hi.astype(np.float32)).astype(F16)
    return hi, lo


def _rounds():
    out = []
    t0 = 0
    while t0 < NT:
        out.append((t0, min(ROUND_LEN, NT - t0)))
        t0 += ROUND_LEN
    return out


def _build_program(use_ylo, use_xlo, ntd):
    _install_wait_split()
    import concourse.bass as bass
    import concourse.mybir as mybir
    from concourse.tile import TileContext
    from contextlib import ExitStack

    dt = mybir.dt
    nc = bass.Bass()
    npool = NT - ntd

    WBITS = nc.declare_dram_parameter("wbits", [128, NT * 128], dt.int16, isOutput=False)
    if npool:
        STP = nc.declare_dram_parameter("stp", [128, npool * 512], dt.int16, isOutput=False)
    # weight pack layout (fp16):
    #   wy hi scaled [0:48], wy lo scaled [48:96] (if ylo)
    #   wy hi unscaled [96:144], wy lo unscaled [144:192] (if ylo)
    #   wx hi: 192 + 65*(3c+t), 12 blocks; wx lo after (if xlo)
    WXO = 192
    WTOT = WXO + 780 + (780 if use_xlo else 0)
    WPACK = nc.declare_dram_parameter("wpack", [128, WTOT], dt.float16, isOutput=False)
    OUT = nc.declare_dram_parameter("outbuf", [65, 3 * 512], dt.float32, isOutput=True)

    rounds = _rounds()

    with ExitStack() as ctx:
        tc = ctx.enter_context(TileContext(nc))
        pp = ctx.enter_context(tc.tile_pool(name="persist", bufs=1))
        pA = ctx.enter_context(tc.tile_pool(name="psA", bufs=6, space="PSUM"))
        pB = ctx.enter_context(tc.tile_pool(name="psB", bufs=2, space="PSUM"))

        wbig = pp.tile([128, NT * 128], dt.int16, name="wbig", tag="wbig")
        stp = (
            pp.tile([128, npool * 512], dt.int16, name="stp", tag="stp")
            if npool
            else None
        )
        wpk = pp.tile([128, WTOT], dt.float16, name="wpk", tag="wpk")
        m = [
            pp.tile([128, NT * 512], dt.int16, name=f"m{t}", tag=f"m{t}")
            for t in range(NTYPES)
        ]
        AT = [
            pp.tile([128, NT * 45], dt.float16, name=f"at{c}", tag=f"at{c}")
            for c in range(4)
        ]
        outp = pp.tile([65, 3 * 512], dt.float32, name="outp", tag="outp")

        if npool:
            nc.sync.dma_start(out=stp[:, :], in_=STP[:, :])
        nc.sync.dma_start(out=wpk[:, :], in_=WPACK[:, :])
        # graded input DMAs so mask passes can start early
        grade = [int(x) for x in os.environ.get("KERNEL_GRADE", "4").split(",")]
        while sum(grade) < ntd:
            grade.append(ntd - sum(grade))
        chunks = []
        pos = 0
        for g in grade:
            end = min(ntd, pos + g)
            if end > pos:
                nc.sync.dma_start(
                    out=wbig[:, 128 * pos: 128 * end],
                    in_=WBITS[:, 128 * pos: 128 * end],
                )
                chunks.append((pos, end))
            pos = end

        # ---- masks ------------------------------------------------------
        # DVE bitvec passes: (dma chunk, slot=x-chunk, type)
        for (c0, c1) in chunks:
            win = wbig[:, 128 * c0: 128 * c1]
            for s in range(4):
                for t in range(NTYPES):
                    p = 4 * s + t
                    dst = m[t].rearrange("p (T sl n) -> p T sl n", sl=4, n=128)[
                        :, c0:c1, s, :
                    ]
                    if p == 10:
                        nc.vector.tensor_scalar(
                            dst, win, MBIT, None, mybir.AluOpType.bitwise_and
                        )
                    elif p < 10:
                        nc.vector.tensor_scalar(
                            dst, win, 10 - p, MBIT,
                            mybir.AluOpType.logical_shift_left,
                            op1=mybir.AluOpType.bitwise_and,
                        )
                    else:
                        nc.vector.tensor_scalar(
                            dst, win, p - 10, MBIT,
                            mybir.AluOpType.logical_shift_right,
                            op1=mybir.AluOpType.bitwise_and,
                        )
        # Pool is_equal masks (value 1.0 fp16) for tiles [ntd, NT)
        for Ti in range(npool):
            T = ntd + Ti
            for t in range(NTYPES):
                nc.gpsimd.tensor_scalar(
                    m[t][:, 512 * T: 512 * T + 512].bitcast(dt.float16),
                    stp[:, 512 * Ti: 512 * Ti + 512],
                    float(t + 1),
                    None,
                    mybir.AluOpType.is_equal,
                )

        # weight views
        wy_hi_s = [wpk[:, 16 * t: 16 * t + 15] for t in range(NTYPES)]
        wy_lo_s = [wpk[:, 48 + 16 * t: 48 + 16 * t + 15] for t in range(NTYPES)]
        wy_hi_u = [wpk[:, 96 + 16 * t: 96 + 16 * t + 15] for t in range(NTYPES)]
        wy_lo_u = [wpk[:, 144 + 16 * t: 144 + 16 * t + 15] for t in range(NTYPES)]
        wx_hi = [
            [wpk[:, WXO + 65 * (3 * c + t): WXO + 65 * (3 * c + t) + 65] for t in range(NTYPES)]
            for c in range(4)
        ]
        wx_lo = (
            [
                [
                    wpk[:, WXO + 780 + 65 * (3 * c + t): WXO + 780 + 65 * (3 * c + t) + 65]
                    for t in range(NTYPES)
                ]
                for c in range(4)
            ]
            if use_xlo
            else None
        )

        # ---- stage A-T + drains + stage B + assembly, round by round ----
        for r, (T0, L) in enumerate(rounds):
            for c in range(4):
                psA = pA.tile([128, 45 * ROUND_LEN], dt.float32, name="psA")
                for Tl in range(L):
                    T = T0 + Tl
                    scaled = T < ntd
                    for t in range(NTYPES):
                        lhs = m[t][:, 512 * T + 128 * c: 512 * T + 128 * c + 128].bitcast(
                            dt.float16
                        )
                        dst = psA[:, 45 * Tl + 15 * t: 45 * Tl + 15 * t + 15]
                        rhs_hi = (wy_hi_s if scaled else wy_hi_u)[t]
                        if use_ylo:
                            nc.tensor.matmul(dst, lhsT=lhs, rhs=rhs_hi, start=True, stop=False)
                            nc.tensor.matmul(
                                dst, lhsT=lhs,
                                rhs=(wy_lo_s if scaled else wy_lo_u)[t],
                                start=False, stop=True,
                            )
                        else:
                            nc.tensor.matmul(dst, lhsT=lhs, rhs=rhs_hi, start=True, stop=True)
                nc.scalar.copy(AT[c][:, 45 * T0: 45 * (T0 + L)], psA[:, 0: 45 * L])
            # stage B for this round: psB cols laid out (t, Tl, q)
            psB = pB.tile([65, 45 * ROUND_LEN], dt.float32, name="psB")
            for t in range(NTYPES):
                dstB = psB[:, 15 * ROUND_LEN * t: 15 * ROUND_LEN * t + 15 * L].rearrange(
                    "p (T q) -> p T q", q=15
                )
                for c in range(4):
                    rhs = AT[c].rearrange("p (T g q) -> p T g q", g=3, q=15)[
                        :, T0: T0 + L, t, :
                    ]
                    nc.tensor.matmul(
                        dstB, lhsT=wx_hi[c][t], rhs=rhs,
                        start=(c == 0), stop=(c == 3 and not use_xlo),
                    )
                if use_xlo:
                    for c in range(4):
                        rhs = AT[c].rearrange("p (T g q) -> p T g q", g=3, q=15)[
                            :, T0: T0 + L, t, :
                        ]
                        nc.tensor.matmul(
                            dstB, lhsT=wx_lo[c][t], rhs=rhs,
                            start=False, stop=(c == 3),
                        )
            # assembly: outp[:, 512t + 15T + q] = 64 - psB[:, (t, Tl, q)]
            nfull = L if T0 + L < NT else L - 1
            for t in range(NTYPES):
                nc.scalar.activation(
                    outp[:, 512 * t + 15 * T0: 512 * t + 15 * (T0 + nfull)],
                    psB[:, 15 * ROUND_LEN * t: 15 * ROUND_LEN * t + 15 * nfull],
                    mybir.ActivationFunctionType.Copy, bias=64.0, scale=-1.0,
                )
                if nfull < L:  # tail tile T=34: only bins 510, 511
                    nc.scalar.activation(
                        outp[:, 512 * t + 510: 512 * t + 512],
                        psB[:, 15 * ROUND_LEN * t + 15 * (L - 1): 15 * ROUND_LEN * t + 15 * (L - 1) + 2],
                        mybir.ActivationFunctionType.Copy, bias=64.0, scale=-1.0,
                    )
            if T0 + L == NT:
                nc.sync.dma_start(out=OUT[:, :], in_=outp[:, :])
    return nc


def _get_program(use_ylo, use_xlo, ntd):
    key = (use_ylo, use_xlo, ntd)
    if key not in _PROG_CACHE:
        _PROG_CACHE[key] = _build_program(use_ylo, use_xlo, ntd)
    return _PROG_CACHE[key]


def kernel(site_type_map, site_size_x, site_size_y):
    from concourse.bass_utils import run_bass_kernel_spmd

    smap = np.asarray(site_type_map, dtype=np.int32)
    sx = np.asarray(site_size_x, dtype=np.float32)
    sy = np.asarray(site_size_y, dtype=np.float32)

    WYf = _build_wy(sy)  # [128, 3, 15]
    WXf = _build_wx(sx)  # [512, 3, 65]
    wy_hi_s, wy_lo_s = _hi_lo16(WYf * np.float32(WYSCALE))
    wy_hi_u, wy_lo_u = _hi_lo16(WYf)
    wx_hi, wx_lo = _hi_lo16(WXf)
    use_ylo = bool(
        np.any(wy_lo_s.astype(np.float32) != 0) or np.any(wy_lo_u.astype(np.float32) != 0)
    )
    use_xlo = bool(np.any(wx_lo.astype(np.float32) != 0))
    ntd = NT - POOL_TILES

    WXO = 192
    WTOT = WXO + 780 + (780 if use_xlo else 0)
    wpk = np.zeros((128, WTOT), F16)
    for t in range(3):
        wpk[:, 16 * t: 16 * t + 15] = wy_hi_s[:, t, :]
        wpk[:, 48 + 16 * t: 48 + 16 * t + 15] = wy_lo_s[:, t, :]
        wpk[:, 96 + 16 * t: 96 + 16 * t + 15] = wy_hi_u[:, t, :]
        wpk[:, 144 + 16 * t: 144 + 16 * t + 15] = wy_lo_u[:, t, :]
        for c in range(4):
            o = WXO + 65 * (3 * c + t)
            wpk[:, o: o + 65] = wx_hi[128 * c: 128 * c + 128, t, :]
            if use_xlo:
                o2 = WXO + 780 + 65 * (3 * c + t)
                wpk[:, o2: o2 + 65] = wx_lo[128 * c: 128 * c + 128, t, :]

    nc = _get_program(use_ylo, use_xlo, ntd)

    in_maps = []
    for c in range(NCORES):
        sjp = np.zeros((JPAD, RPAD), np.int16)
        sjp[8: 8 + 4096, :] = smap[512 * c: 512 * c + 512].T
        # 3-bit planes then nibble-pack 4 sites (x slots) per word
        bits = (
            (sjp == 1).astype(np.uint16)
            | ((sjp == 2).astype(np.uint16) << 1)
            | ((sjp == 3).astype(np.uint16) << 2)
        )
        wbits = np.empty((128, NT * 128), np.int16)
        stp_l = []
        for T in range(NT):
            blk = bits[120 * T: 120 * T + 128, :].reshape(128, 4, 128)
            w = (
                blk[:, 0, :]
                | (blk[:, 1, :] << 4)
                | (blk[:, 2, :] << 8)
                | (blk[:, 3, :] << 12)
            )
            wbits[:, 128 * T: 128 * T + 128] = w.view(np.int16)
            if T >= ntd:
                stp_l.append(sjp[120 * T: 120 * T + 128, :])
        mm = {"wbits": wbits, "wpack": wpk}
        if stp_l:
            mm["stp"] = np.concatenate(stp_l, axis=1)
        in_maps.append(mm)

    res = run_bass_kernel_spmd(
        nc,
        in_maps,
        core_ids=list(range(NCORES)),
        trace=bool(int(os.environ.get("KERNEL_TRACE", "0"))),
    )
    kernel._last_results = res

    # device returns 3 type planes; comp2site=(1,1,2,3) duplicates plane 0
    full = np.empty((4, 512, 512), np.float32)
    for c in range(NCORES):
        ob = res.results[c]["outbuf"]  # [65, 1536]
        for t in range(3):
            full[t + 1, 64 * c: 64 * c + 64, :] = ob[0:64, 512 * t: 512 * t + 512]
    for c in range(NCORES - 1):
        ob = res.results[c]["outbuf"]
        for t in range(3):
            # spill row: (64-h0) + (64-h1) - 64 = 64 - h0 - h1
            full[t + 1, 64 * (c + 1), :] += ob[64, 512 * t: 512 * t + 512] - np.float32(64.0)
    full[0] = full[1]
    return full


# revision 14
# speedup vs baseline: 1.6345x; 1.4170x over previous
"""Trainium2 Bass kernel for nn_DemandMap (histogram_binning).

Math: the scatter-add histogram is a dense separable 8x8 block reduction:
hist_t = WX_t^T @ mask_t @ WY_t with banded weights by (residue, type).

Stage A runs *transposed* on the PE (mask is the stationary operand), so no
DMA transpose and only narrow PSUM drains are needed:

  input: per j-tile T and word n, an int16 packing the 3 mask bits of TWO
         sites: bit (2+2t) = [type(x=2n)==t+1], bit (10+2t) = [type(x=2n+1)
         ==t+1].  One DVE pass per (tile block, type) -- (>>2t, &0x0404),
         both bitvec ops, all operands 2-byte so the 4x DVE mode applies --
         yields bytes in {0x00, 0x04}; bitcast to fp8e5m2 that is exactly
         2^-14 * mask, with wy pre-scaled by 2^14 (exact).  The last
         POOL_TILES j-tiles instead get is_equal masks (value 1.0, fp16) on
         the gpsimd engine from a raw site-type side plane, with unscaled wy.
  stage A-T: psAT[i, (T,t,q)] = mask_t[:, x-chunk].T @ wy_t -- the moving
         operand is only 15 columns so PE cost is tiny (and LDWEIGHTS is
         free); the output is already x-major.
  drain: psAT f32 -> fp16 AT (exact: values are 0.5-step <= 40), spread
         across ACT/DVE/Pool.
  stage B: psB[65, (t,T,q)] += wx_c^T @ AT_c accumulated over the 4 x-chunks.
  assembly: 64 - psB into the output planes; row 64 is the kx=1 spill into
         the next core's first x-bin, merged on the host.  Output DMA per
         round.

Sharding: core c owns x rows [512c, 512c+512) and bins bx in [64c, 64c+64).
"""

import json
import os

import numpy as np

F16 = np.float16

NCORES = 8
RPAD = 512   # site x cols per core
JPAD = 4224  # 8 top zero pad + 4096 + tail pad (y axis, partition source)
NT = 35      # j-tiles, stride 120, each owns 15 y-bins
NTYPES = 3
PAIR_AND = 0x0404    # surviving bits -> fp8e5m2 2^-14 in each byte
WYSCALE = 2.0 ** 14

POOL_TILES = int(os.environ.get("KERNEL_POOL_TILES", "3"))
ROUNDS = [int(x) for x in os.environ.get("KERNEL_ROUNDS", "5,10,10,10").split(",")]
assert sum(ROUNDS) == NT and all(l <= 11 for l in ROUNDS)
# engine for drain (r, c): A=ACT, D=DVE (Pool cannot access PSUM)
DRAIN_ENG = os.environ.get("KERNEL_DRAIN", "AAAA,ADAA,ADAA,AAAA").split(",")

_PROG_CACHE = {}
_WSPLIT_DONE = [False]


def _install_wait_split():
    """This walrus build accepts only ONE sync wait per instruction; Tile
    attaches N.  Rewrite the BIR JSON: hoist all-but-one wait onto fresh
    same-engine EventSemaphore waits inserted before the offender."""
    if _WSPLIT_DONE[0]:
        return
    import concourse.bass as bass

    orig = bass.Bass.to_json_bytes

    def split(self, *a, **k):
        m = json.loads(orig(self, *a, **k))
        n = [0]
        for fn in m["functions"]:
            for blk in fn["blocks"]:
                out = []
                for ins in blk["instructions"]:
                    si = ins.get("sync_info")
                    waits = si.get("on_wait") if si else None
                    if waits and len(waits) > 1:
                        for w in waits[:-1]:
                            n[0] += 1
                            nop = {
                                "engine": ins["engine"],
                                "ins": [],
                                "outs": [],
                                "name": f"WSPLIT-{n[0]}",
                                "opcode": "EventSemaphore",
                                "sync_info": {"on_update": [], "on_wait": [w]},
                            }
                            if "debug" in ins:
                                nop["debug"] = ins["debug"]
                            out.append(nop)
                        si["on_wait"] = [waits[-1]]
                    out.append(ins)
                blk["instructions"] = out
        return json.dumps(m).encode()

    bass.Bass.to_json_bytes = split
    _WSPLIT_DONE[0] = True


def _oxy_weights(size_f32):
    """Per-residue overlap weights, matching the reference f32 formulas."""
    r = np.arange(8, dtype=np.float32)
    o0 = np.maximum(np.minimum(np.float32(8.0) - r, size_f32), np.float32(0.0))
    o1 = np.maximum(
        np.minimum(np.float32(16.0) - r, size_f32) - (np.float32(8.0) - r),
        np.float32(0.0),
    )
    return o0.astype(np.float32), o1.astype(np.float32)


def _build_wy(sy):
    """f32 [128, 3, 15]: y-stage stationary (y-halo tile form), types 1..3.
    Tile partition p holds padded j = 120T + p (jp = j + 8), so ry = p % 8;
    bin q gets ky=0 rows p//8 == q+1 and ky=1 rows p//8 == q."""
    W = np.zeros((128, 3, 15), np.float32)
    for tp in range(3):
        o0, o1 = _oxy_weights(np.float32(sy[tp + 1]))
        for p in range(128):
            if 0 <= p // 8 - 1 < 15:
                W[p, tp, p // 8 - 1] += o0[p % 8]
            if p // 8 < 15:
                W[p, tp, p // 8] += o1[p % 8]
    return W


def _build_wx(sx):
    """f32 [512, 3, 65]: x-stage stationary.  Local row r = x - 512c; col 64
    is the spill bin (kx=1 of the last 8 rows -> next core's first bin)."""
    W = np.zeros((RPAD, 3, 65), np.float32)
    for tp in range(3):
        o0, o1 = _oxy_weights(np.float32(sx[tp + 1]))
        for r in range(512):
            W[r, tp, r // 8] += o0[r % 8]
            W[r, tp, r // 8 + 1] += o1[r % 8]
    return W


def _hi_lo16(w):
    hi = w.astype(F16)
    lo = (w - hi.astype(np.float32)).astype(F16)
    return hi, lo


def _build_program(use_ylo, use_xlo, ntd):
    _install_wait_split()
    import concourse.bass as bass
    import concourse.mybir as mybir
    from concourse.tile import TileContext
    from contextlib import ExitStack

    dt = mybir.dt
    nc = bass.Bass()
    npool = NT - ntd
    Lmax = max(ROUNDS)

    WBITS = nc.declare_dram_parameter("wbits", [128, ntd * 256], dt.int16, isOutput=False)
    if npool:
        STP = nc.declare_dram_parameter("stp", [128, npool * 512], dt.int16, isOutput=False)
    # weight pack (fp16): wy hi scaled [0:48], wy lo scaled [48:96],
    # wy hi unscaled [96:144], wy lo unscaled [144:192],
    # wx hi 192+65*(3c+t) (negated); wx lo after (if xlo);
    # then w64 [65] = 0.5 and wones [15*Lmax] = 1.0 for the +64 bias matmul
    WXO = 192
    WXE = WXO + 780 + (780 if use_xlo else 0)
    WTOT = WXE + 65 + 15 * Lmax
    WPACK = nc.declare_dram_parameter("wpack", [128, WTOT], dt.float16, isOutput=False)
    OUT = nc.declare_dram_parameter("outbuf", [65, 3 * 512], dt.float32, isOutput=True)

    with ExitStack() as ctx:
        tc = ctx.enter_context(TileContext(nc))
        pp = ctx.enter_context(tc.tile_pool(name="persist", bufs=1))
        pA = ctx.enter_context(tc.tile_pool(name="psA", bufs=6, space="PSUM"))
        pB = ctx.enter_context(tc.tile_pool(name="psB", bufs=2, space="PSUM"))

        wbig = pp.tile([128, ntd * 256], dt.int16, name="wbig", tag="wbig")
        stp = (
            pp.tile([128, npool * 512], dt.int16, name="stp", tag="stp")
            if npool
            else None
        )
        wpk = pp.tile([128, WTOT], dt.float16, name="wpk", tag="wpk")
        m = [
            pp.tile([128, ntd * 256], dt.int16, name=f"m{t}", tag=f"m{t}")
            for t in range(NTYPES)
        ]
        mp = [
            pp.tile([128, npool * 512], dt.float16, name=f"mp{t}", tag=f"mp{t}")
            for t in range(NTYPES)
        ] if npool else None
        AT = [
            pp.tile([128, NT * 45], dt.float16, name=f"at{c}", tag=f"at{c}")
            for c in range(4)
        ]
        outp = pp.tile([65, 3 * 512], dt.float32, name="outp", tag="outp")

        # ---- DMAs: round-0 bits first, then pool plane + weights ---------
        bounds = np.cumsum([0] + ROUNDS)
        dve_rng = []  # DVE tile range per round
        for r in range(len(ROUNDS)):
            dve_rng.append((bounds[r], min(bounds[r + 1], ntd)))
        r0a, r0b = dve_rng[0]
        nc.sync.dma_start(out=wbig[:, 256 * r0a: 256 * r0b], in_=WBITS[:, 256 * r0a: 256 * r0b])
        if npool:
            nc.sync.dma_start(out=stp[:, :], in_=STP[:, :])
        nc.sync.dma_start(out=wpk[:, :], in_=WPACK[:, :])
        for r in range(1, len(ROUNDS)):
            a, b = dve_rng[r]
            if b > a:
                nc.sync.dma_start(out=wbig[:, 256 * a: 256 * b], in_=WBITS[:, 256 * a: 256 * b])

        # ---- masks -------------------------------------------------------
        # Pool is_equal masks (value 1.0 fp16) for tiles [ntd, NT)
        for Ti in range(npool):
            for t in range(NTYPES):
                nc.gpsimd.tensor_scalar(
                    mp[t][:, 512 * Ti: 512 * Ti + 512],
                    stp[:, 512 * Ti: 512 * Ti + 512],
                    float(t + 1),
                    None,
                    mybir.AluOpType.is_equal,
                )

        def dve_block(r):
            a, b = dve_rng[r]
            if b <= a:
                return
            win = wbig[:, 256 * a: 256 * b]
            for t in range(NTYPES):
                dst = m[t][:, 256 * a: 256 * b]
                if t == 0:
                    nc.vector.tensor_scalar(
                        dst, win, PAIR_AND, None, mybir.AluOpType.bitwise_and
                    )
                else:
                    nc.vector.tensor_scalar(
                        dst, win, 2 * t, PAIR_AND,
                        mybir.AluOpType.logical_shift_right,
                        op1=mybir.AluOpType.bitwise_and,
                    )

        # weight views
        wy_hi_s = [wpk[:, 16 * t: 16 * t + 15] for t in range(NTYPES)]
        wy_lo_s = [wpk[:, 48 + 16 * t: 48 + 16 * t + 15] for t in range(NTYPES)]
        wy_hi_u = [wpk[:, 96 + 16 * t: 96 + 16 * t + 15] for t in range(NTYPES)]
        wy_lo_u = [wpk[:, 144 + 16 * t: 144 + 16 * t + 15] for t in range(NTYPES)]
        wx_hi = [
            [wpk[:, WXO + 65 * (3 * c + t): WXO + 65 * (3 * c + t) + 65] for t in range(NTYPES)]
            for c in range(4)
        ]
        wx_lo = (
            [
                [
                    wpk[:, WXO + 780 + 65 * (3 * c + t): WXO + 780 + 65 * (3 * c + t) + 65]
                    for t in range(NTYPES)
                ]
                for c in range(4)
            ]
            if use_xlo
            else None
        )

        w64 = wpk[:, WXE: WXE + 65]
        wones = wpk[:, WXE + 65: WXE + 65 + 15 * Lmax]

        def drain(eng, dst, src):
            if eng == "A":
                nc.scalar.copy(dst, src)
            else:
                nc.vector.tensor_copy(out=dst, in_=src)

        # emit DVE blocks 0,1 up front; later blocks interleave after rounds
        dve_block(0)
        dve_block(1)

        for r, L in enumerate(ROUNDS):
            T0 = bounds[r]
            for c in range(4):
                psA = pA.tile([128, 45 * Lmax], dt.float32, name="psA")
                for Tl in range(L):
                    T = T0 + Tl
                    for t in range(NTYPES):
                        if T < ntd:
                            lhs = m[t][:, 256 * T + 64 * c: 256 * T + 64 * c + 64].bitcast(
                                dt.float8e5
                            )
                            rh, rl = wy_hi_s[t], wy_lo_s[t]
                        else:
                            Ti = T - ntd
                            lhs = mp[t][:, 512 * Ti + 128 * c: 512 * Ti + 128 * c + 128]
                            rh, rl = wy_hi_u[t], wy_lo_u[t]
                        dst = psA[:, 45 * Tl + 15 * t: 45 * Tl + 15 * t + 15]
                        if use_ylo:
                            nc.tensor.matmul(dst, lhsT=lhs, rhs=rh, start=True, stop=False)
                            nc.tensor.matmul(dst, lhsT=lhs, rhs=rl, start=False, stop=True)
                        else:
                            nc.tensor.matmul(dst, lhsT=lhs, rhs=rh, start=True, stop=True)
                drain(
                    DRAIN_ENG[r][c],
                    AT[c][:, 45 * T0: 45 * (T0 + L)],
                    psA[:, 0: 45 * L],
                )
            if r + 2 < len(ROUNDS):
                dve_block(r + 2)
            # stage B: psB cols (t, Tl, q)
            psB = pB.tile([65, 45 * Lmax], dt.float32, name="psB")
            for t in range(NTYPES):
                dstB = psB[:, 15 * Lmax * t: 15 * Lmax * t + 15 * L].rearrange(
                    "p (T q) -> p T q", q=15
                )
                for c in range(4):
                    rhs = AT[c].rearrange("p (T g q) -> p T g q", g=3, q=15)[
                        :, T0: T0 + L, t, :
                    ]
                    nc.tensor.matmul(
                        dstB, lhsT=wx_hi[c][t], rhs=rhs,
                        start=(c == 0), stop=False,
                    )
                if use_xlo:
                    for c in range(4):
                        rhs = AT[c].rearrange("p (T g q) -> p T g q", g=3, q=15)[
                            :, T0: T0 + L, t, :
                        ]
                        nc.tensor.matmul(
                            dstB, lhsT=wx_lo[c][t], rhs=rhs,
                            start=False, stop=False,
                        )
                # +64 bias (wx is negated): psB = 64 - hist
                nc.tensor.matmul(
                    dstB, lhsT=w64, rhs=wones[:, 0: 15 * L].rearrange(
                        "p (T q) -> p T q", q=15
                    ),
                    start=False, stop=True,
                )
            # stage psB (already 64 - hist) to SBUF, then per-round DMA out
            nfull = L if T0 + L < NT else L - 1
            nc.scalar.copy(
                outp.rearrange("p (g y) -> p g y", y=512)[
                    :, :, 15 * T0: 15 * (T0 + nfull)
                ],
                psB.rearrange("p (g x) -> p g x", x=15 * Lmax)[:, :, 0: 15 * nfull],
            )
            if nfull < L:  # tail tile T=34: only bins 510, 511
                nc.scalar.copy(
                    outp.rearrange("p (g y) -> p g y", y=512)[:, :, 510:512],
                    psB.rearrange("p (g x) -> p g x", x=15 * Lmax)[
                        :, :, 15 * (L - 1): 15 * (L - 1) + 2
                    ],
                )
            lastc = 512 if T0 + L == NT else 15 * (T0 + nfull)
            nc.sync.dma_start(
                out=OUT.rearrange("p (g y) -> p g y", y=512)[:, :, 15 * T0: lastc],
                in_=outp.rearrange("p (g y) -> p g y", y=512)[:, :, 15 * T0: lastc],
            )
    return nc


def _get_program(use_ylo, use_xlo, ntd):
    key = (use_ylo, use_xlo, ntd)
    if key not in _PROG_CACHE:
        _PROG_CACHE[key] = _build_program(use_ylo, use_xlo, ntd)
    return _PROG_CACHE[key]


def kernel(site_type_map, site_size_x, site_size_y):
    from concourse.bass_utils import run_bass_kernel_spmd

    smap = np.asarray(site_type_map, dtype=np.int32)
    sx = np.asarray(site_size_x, dtype=np.float32)
    sy = np.asarray(site_size_y, dtype=np.float32)

    WYf = _build_wy(sy)  # [128, 3, 15]
    WXf = _build_wx(sx)  # [512, 3, 65]
    wy_hi_s, wy_lo_s = _hi_lo16(WYf * np.float32(WYSCALE))
    wy_hi_u, wy_lo_u = _hi_lo16(WYf)
    wx_hi, wx_lo = _hi_lo16(-WXf)  # negated: psB accumulates 64 - hist
    use_ylo = bool(
        np.any(wy_lo_s.astype(np.float32) != 0) or np.any(wy_lo_u.astype(np.float32) != 0)
    )
    use_xlo = bool(np.any(wx_lo.astype(np.float32) != 0))
    ntd = NT - POOL_TILES
    Lmax = max(ROUNDS)

    WXO = 192
    WXE = WXO + 780 + (780 if use_xlo else 0)
    WTOT = WXE + 65 + 15 * Lmax
    wpk = np.zeros((128, WTOT), F16)
    for t in range(3):
        wpk[:, 16 * t: 16 * t + 15] = wy_hi_s[:, t, :]
        wpk[:, 48 + 16 * t: 48 + 16 * t + 15] = wy_lo_s[:, t, :]
        wpk[:, 96 + 16 * t: 96 + 16 * t + 15] = wy_hi_u[:, t, :]
        wpk[:, 144 + 16 * t: 144 + 16 * t + 15] = wy_lo_u[:, t, :]
        for c in range(4):
            o = WXO + 65 * (3 * c + t)
            wpk[:, o: o + 65] = wx_hi[128 * c: 128 * c + 128, t, :]
            if use_xlo:
                o2 = WXO + 780 + 65 * (3 * c + t)
                wpk[:, o2: o2 + 65] = wx_lo[128 * c: 128 * c + 128, t, :]
    wpk[:, WXE: WXE + 65] = np.float16(0.5)       # w64: 128 * 0.5 = 64
    wpk[:, WXE + 65: WXE + 65 + 15 * Lmax] = np.float16(1.0)  # wones

    nc = _get_program(use_ylo, use_xlo, ntd)

    in_maps = []
    for c in range(NCORES):
        sjp = np.zeros((JPAD, RPAD), np.int16)
        sjp[8: 8 + 4096, :] = smap[512 * c: 512 * c + 512].T
        wbits = np.empty((128, ntd * 256), np.int16)
        stp_l = []
        for T in range(NT):
            blk = sjp[120 * T: 120 * T + 128, :]
            if T < ntd:
                w = np.zeros((128, 256), np.uint16)
                for t in (1, 2, 3):
                    mk = (blk == t).astype(np.uint16).reshape(128, 256, 2)
                    w |= (mk[:, :, 0] << (2 * t)) | (mk[:, :, 1] << (8 + 2 * t))
                wbits[:, 256 * T: 256 * T + 256] = w.view(np.int16)
            else:
                stp_l.append(blk)
        mm = {"wbits": wbits, "wpack": wpk}
        if stp_l:
            mm["stp"] = np.concatenate(stp_l, axis=1)
        in_maps.append(mm)

    res = run_bass_kernel_spmd(
        nc,
        in_maps,
        core_ids=list(range(NCORES)),
        trace=bool(int(os.environ.get("KERNEL_TRACE", "0"))),
    )
    kernel._last_results = res

    # device returns 3 type planes; comp2site=(1,1,2,3) duplicates plane 0
    full = np.empty((4, 512, 512), np.float32)
    for c in range(NCORES):
        ob = res.results[c]["outbuf"]  # [65, 1536]
        for t in range(3):
            full[t + 1, 64 * c: 64 * c + 64, :] = ob[0:64, 512 * t: 512 * t + 512]
    for c in range(NCORES - 1):
        ob = res.results[c]["outbuf"]
        for t in range(3):
            # spill row: (64-h0) + (64-h1) - 64 = 64 - h0 - h1
            full[t + 1, 64 * (c + 1), :] += ob[64, 512 * t: 512 * t + 512] - np.float32(64.0)
    full[0] = full[1]
    return full
